# revision 1
# baseline (speedup 1.0000x reference)
"""Trainium2 Bass kernel for a custom Jacobi-basis layer.

Math:
    t = tanh(x)                                  x: [B, I] f32
    J[b,i,k] = P_k^(1,1)(t[b,i])                 Jacobi polys, k = 0..8
    out[b,o] = sum_{i,k} J[b,i,k] * coeff[o,i,k] * weights[o,i]

Strategy (8 NeuronCores, data-parallel over batch):
  * Fold weights into coeff on host: Cw[o,i,k] = coeff[o,i,k]*weights[o,i].
  * alpha=beta=1 makes the three-term recurrence two-term coefficient-free
    after rescaling: G_1 = t, G_k = t*G_{k-1} - B'_k*G_{k-2} with G_k = c_k*J_k.
    The 1/c_k scale is folded into the (host-prepared) matmul operand.
  * J_0 == 1, so the k=0 term is a per-output bias, applied with a K=1 matmul.
  * Per core: tanh/square on ScalarE, fp32 recurrence on VectorE (fused
    scalar_tensor_tensor ops, L/R half chains), one fp16 rounding cast per
    plane chunk on ScalarE, then 128 fp16 matmuls [128x128]@[128x512]
    accumulating fp32 in PSUM over the 4096-long (i,k) contraction.
    fp16 matmul error is ~3e-4 (vs 2.3e-3 bf16) and runs at full PE rate.
    Keeping the recurrence itself in fp32 avoids error compounding over k
    (a bf16 recurrence measures 2e-2; this pipeline measures ~3.6e-4).
  * DMA priority ladder: consts -> xt halves -> r planes (depth-2), so the
    tanh/recurrence/matmul pipeline starts as early as possible; PE is
    HAM-warmed with memset-sourced junk matmuls before the real stream.
"""

import numpy as np

import concourse.mybir as mybir
import concourse.tile as tile
from concourse import bacc
from concourse.bass_utils import run_bass_kernel_spmd

ORDER = 8
B, I, O = 4096, 512, 512
NCORES = 8
BC = B // NCORES          # batch rows per core = 512
P = 128                   # partitions
NIC = I // P              # i-chunks = 4
BT = BC // P              # b-tiles per core = 4
FREE = NIC * BC           # free dim of basis planes = 2048


def _consts():
    """Recurrence constants (alpha=beta=1, so the k2 term is 0)."""
    a = b = 1.0
    A, Bk = {}, {}
    for i in range(2, ORDER + 1):
        A[i] = (2 * i + a + b) * (2 * i + a + b - 1) / (2 * i * (i + a + b))
        Bk[i] = (i + a - 1) * (i + b - 1) * (2 * i + a + b) / (
            i * (i + a + b) * (2 * i + a + b - 2)
        )
    c = {0: 1.0, 1: 0.5}
    for i in range(2, ORDER + 1):
        c[i] = c[i - 1] / A[i]
    Bp = {i: Bk[i] * c[i] / c[i - 2] for i in range(2, ORDER + 1)}
    return c, Bp


def _build_module():
    nc = bacc.Bacc("TRN2", num_devices=NCORES)
    f32 = mybir.dt.float32
    f16 = mybir.dt.float16

    # xt stored half-major: [h, p, H] so each half is one contiguous DMA
    xt_d = nc.dram_tensor("xt", [2, P, FREE // 2], f32, kind="ExternalInput")
    # r layout: [p, (k-1)*FREE + ic*O + o] = Cw[o, ic*128+p, k] / c_k
    r_d = nc.dram_tensor("r", [P, ORDER * FREE], f16, kind="ExternalInput")
    # consts row 0 = [ones(128) | bias(512)]; rows 1..127 warmup junk
    consts_d = nc.dram_tensor("consts", [P, P + O], f16, kind="ExternalInput")
    # out layout: [p, bt*O + o] = output[core*BC + bt*128 + p, o]
    out_d = nc.dram_tensor("out", [P, BT * O], f32, kind="ExternalOutput")

    _, Bp = _consts()
    mult = mybir.AluOpType.mult
    add = mybir.AluOpType.add

    from concourse.tile_rust import add_dep_helper

    with tile.TileContext(nc) as tc:
        with (
            tc.tile_pool(name="io", bufs=1) as io,
            tc.tile_pool(name="g", bufs=1) as gp,
            tc.tile_pool(name="u", bufs=2) as up,
            tc.tile_pool(name="psum", bufs=1, space="PSUM") as pp,
        ):
            def chunk(ap, ic):
                return ap[:, ic * BC : (ic + 1) * BC]

            # consts first (tiny; also feeds the PE warmup), then xt in four
            # chained ic-chunks, then the r planes on a depth-2 ladder.
            const_t = io.tile([P, P + O], f16, tag="consts")
            nc.sync.dma_start(const_t[:], consts_d[:])
            ones_t = const_t[0:1, 0:P]
            bias_t = const_t[0:1, P : P + O]
            x_t = io.tile([P, FREE], f32, tag="x")
            H = FREE // 2
            d_xl = nc.sync.dma_start(x_t[:, 0:H], xt_d[0])
            d_prev_x = nc.sync.dma_start(x_t[:, H:FREE], xt_d[1])
            add_dep_helper(d_prev_x.ins, d_xl.ins, reason="dma ladder")
            # r planes ladder behind the xt halves (xt gates the whole
            # compute pipeline; r_k is only needed when PE reaches plane k).
            r_t = []
            d_prev = [None, d_prev_x]
            for k in range(ORDER):
                rt = io.tile([P, FREE], f16, tag=f"r{k}", name=f"r{k}")
                d = nc.sync.dma_start(rt[:], r_d[:, k * FREE : (k + 1) * FREE])
                if d_prev[k % 2] is not None:
                    add_dep_helper(d.ins, d_prev[k % 2].ins, reason="dma ladder")
                d_prev[k % 2] = d
                r_t.append(rt)

            # Basis planes G_1..G_8: recurrence in fp32 on VectorE at per-ic
            # granularity (the four ic-chunks are independent chains), each
            # chunk rounded to fp16 on ScalarE for the matmuls. G_8 is written
            # in fp16 directly (nothing downstream needs it in fp32).
            g = [None] * (ORDER + 1)
            gr = [None] * (ORDER + 1)

            t = gp.tile([P, FREE], f32, tag="t")
            sq = up.tile([P, FREE], f32, tag="sq")
            gr[1] = gp.tile([P, FREE], f16, tag="gr", name="gr1", bufs=4)
            for ic in range(NIC):
                nc.scalar.activation(
                    chunk(t, ic), chunk(x_t, ic),
                    mybir.ActivationFunctionType.Tanh,
                )
                nc.scalar.square(chunk(sq, ic), chunk(t, ic))
                nc.scalar.copy(chunk(gr[1], ic), chunk(t, ic))
            g[1] = t
            # g2 = s - B2 on ScalarE (off the DVE chain)
            g2 = gp.tile([P, FREE], f32, tag="g", name="g2", bufs=3)
            gr[2] = gp.tile([P, FREE], f16, tag="gr", name="gr2", bufs=4)
            for ic in range(NIC):
                nc.scalar.activation(
                    chunk(g2, ic), chunk(sq, ic),
                    mybir.ActivationFunctionType.Copy, bias=-Bp[2],
                )
                nc.scalar.copy(chunk(gr[2], ic), chunk(g2, ic))
            g[2] = g2

            # DVE chain at L/R half granularity (lower per-op overhead; the
            # two halves are independent chains). u3 = (s - B2)*t skips g2.
            halves = (slice(0, H), slice(H, FREE))
            u3 = up.tile([P, FREE], f32, tag="u", name="u3")
            g3 = gp.tile([P, FREE], f32, tag="g", name="g3", bufs=3)
            gr[3] = gp.tile([P, FREE], f16, tag="gr", name="gr3", bufs=4)
            for h in (0, 1):
                sl = halves[h]
                nc.vector.scalar_tensor_tensor(
                    u3[:, sl], sq[:, sl], -Bp[2], t[:, sl], add, mult
                )
            for h in (0, 1):
                sl = halves[h]
                nc.vector.scalar_tensor_tensor(
                    g3[:, sl], t[:, sl], -Bp[3], u3[:, sl], mult, add
                )
                for ic in (0, 1) if h == 0 else (2, 3):
                    nc.scalar.copy(chunk(gr[3], ic), chunk(g3, ic))
            g[3] = g3
            for k in range(4, ORDER + 1):
                u = up.tile([P, FREE], f32, tag="u", name=f"u{k}")
                last = k == ORDER
                gk = (
                    gp.tile([P, FREE], f16, tag="gr", name=f"g{k}", bufs=4)
                    if last
                    else gp.tile([P, FREE], f32, tag="g", name=f"g{k}", bufs=3)
                )
                if not last:
                    gr[k] = gp.tile(
                        [P, FREE], f16, tag="gr", name=f"gr{k}", bufs=4
                    )
                for h in (0, 1):
                    sl = halves[h]
                    nc.vector.tensor_tensor(
                        u[:, sl], t[:, sl], g[k - 1][:, sl], mult
                    )
                for h in (0, 1):
                    sl = halves[h]
                    nc.vector.scalar_tensor_tensor(
                        gk[:, sl], g[k - 2][:, sl], -Bp[k], u[:, sl], mult, add
                    )
                    if not last:
                        for ic in (0, 1) if h == 0 else (2, 3):
                            nc.scalar.copy(chunk(gr[k], ic), chunk(gk, ic))
                g[k] = gk
                if last:
                    gr[k] = gk

            # Matmuls: psum[bt] = ones^T @ bias + sum_{k,ic} G_k_slice^T @ R_k_slice
            psums = [
                pp.tile([P, O], f32, tag=f"ps{bt}", name=f"ps{bt}")
                for bt in range(BT)
            ]
            # HAM warmup with real K=128 matmuls on the consts block so the
            # clock gate is released before the real stream begins.
            ps_warm = pp.tile([P, O], f32, tag="warm", name="ps_warm")
            warm_t = io.tile([P, P + O], f16, tag="warmsrc")
            nc.vector.memset(warm_t[:], 0.25)
            for w in range(10):
                nc.tensor.matmul(
                    ps_warm[:],
                    warm_t[:, 0:P],
                    warm_t[:, P : P + O],
                    start=True,
                    stop=True,
                )
            for bt in range(BT):
                nc.tensor.matmul(
                    psums[bt][:], ones_t, bias_t, start=True, stop=False
                )
            out_t = io.tile([P, BT * O], f32, tag="out")
            for k in range(1, ORDER + 1):
                if k < ORDER:
                    for ic in range(NIC):
                        for bt in range(BT):
                            col = ic * BC + bt * P
                            nc.tensor.matmul(
                                psums[bt][:],
                                gr[k][:, col : col + P],
                                r_t[k - 1][:, ic * O : (ic + 1) * O],
                                start=False,
                                stop=False,
                            )
                else:
                    # last block: finish b-tiles one at a time so the psum
                    # evictions/stores overlap the remaining matmuls
                    for bt in range(BT):
                        for ic in range(NIC):
                            col = ic * BC + bt * P
                            nc.tensor.matmul(
                                psums[bt][:],
                                gr[k][:, col : col + P],
                                r_t[k - 1][:, ic * O : (ic + 1) * O],
                                start=False,
                                stop=ic == NIC - 1,
                            )
                        dst = out_t[:, bt * O : (bt + 1) * O]
                        if bt % 2 == 0:
                            nc.scalar.copy(dst, psums[bt][:])
                        else:
                            nc.vector.tensor_copy(dst, psums[bt][:])
                        nc.sync.dma_start(
                            out_d[:, bt * O : (bt + 1) * O],
                            out_t[:, bt * O : (bt + 1) * O],
                        )
    nc.compile()
    return nc


def _prep_operands(weights, coeff):
    """Host-side, input-independent preprocessing of the layer constants."""
    c, _ = _consts()
    Cw = coeff.astype(np.float64) * weights.astype(np.float64)[:, :, None]
    bias = Cw[:, :, 0].sum(axis=1)                      # [O]
    r = np.empty((ORDER, P, FREE), dtype=np.float32)
    for k in range(1, ORDER + 1):
        tmp = (Cw[:, :, k] / c[k]).T.astype(np.float32)  # [I, O]
        r[k - 1] = tmp.reshape(NIC, P, O).transpose(1, 0, 2).reshape(P, FREE)
    r = np.ascontiguousarray(
        r.transpose(1, 0, 2).reshape(P, ORDER * FREE)
    ).astype(np.float16)
    consts = np.ones((P, P + O), dtype=np.float32)
    consts[0, P:] = bias
    consts[1:, :] = 0.5
    return r, consts.astype(np.float16)


def _prep_x(x):
    """Per-core [128, FREE] views of x^T: xt[p, ic*BC + b] = x[core*BC+b, ic*128+p]."""
    shards = []
    for core in range(NCORES):
        xc = np.ascontiguousarray(x[core * BC : (core + 1) * BC, :].T)  # [I, BC]
        flat = xc.reshape(NIC, P, BC).transpose(1, 0, 2).reshape(P, FREE)
        shards.append(
            np.ascontiguousarray(
                flat.reshape(P, 2, FREE // 2).transpose(1, 0, 2)
            )
        )
    return shards


def _install_ntff_hook():
    """Register the NTFF profile hook that the image's boot skips (no
    antenv.axon_hooks module). Same ctypes ABI as trn_boot's
    _ntff_profile_via_ctypes. Only used for traced (profiling) runs."""
    import sys
    import types
    import ctypes
    import contextlib

    if "antenv.axon_hooks" in sys.modules:
        return
    mod = types.ModuleType("antenv.axon_hooks")
    state = {"hook": None}
    mod.set_axon_ntff_profile_hook = lambda h: state.__setitem__("hook", h)
    mod.get_axon_ntff_profile_hook = lambda: state["hook"]
    sys.modules["antenv.axon_hooks"] = mod
    import antenv

    antenv.axon_hooks = mod

    so_path = "/opt/axon/libaxon_pjrt.so"
    lib = ctypes.CDLL(so_path)
    if not hasattr(lib, "axon_start_nrt_profile"):
        return
    lib.axon_start_nrt_profile.argtypes = [
        ctypes.POINTER(ctypes.c_int64),
        ctypes.c_size_t,
    ]
    lib.axon_start_nrt_profile.restype = ctypes.c_int64
    lib.axon_stop_nrt_profile.argtypes = [ctypes.c_char_p]
    lib.axon_stop_nrt_profile.restype = ctypes.c_int64

    @contextlib.contextmanager
    def _hook(output_dir, device_ids):
        import jax

        jax.devices()
        if device_ids:
            ids = (ctypes.c_int64 * len(device_ids))(*device_ids)
            rc = lib.axon_start_nrt_profile(ids, len(device_ids))
        else:
            rc = lib.axon_start_nrt_profile(None, 0)
        if rc != 0:
            raise RuntimeError(f"axon_start_nrt_profile rc={rc}")
        try:
            yield
        finally:
            n = lib.axon_stop_nrt_profile(str(output_dir).encode())
            print(f"ntff profile: {n} file(s) written to {output_dir}")

    mod.set_axon_ntff_profile_hook(_hook)


_NC_CACHE = None


def _get_module():
    global _NC_CACHE
    if _NC_CACHE is None:
        _NC_CACHE = _build_module()
    return _NC_CACHE


def _run(x, weights, coeff, trace=False):
    nc = _get_module()
    r, consts = _prep_operands(weights, coeff)
    xs = _prep_x(np.asarray(x, dtype=np.float32))
    in_maps = [
        {"xt": xs[core], "r": r, "consts": consts} for core in range(NCORES)
    ]
    try:
        res = run_bass_kernel_spmd(
            nc, in_maps, core_ids=list(range(NCORES)), trace=trace
        )
    except Exception:
        res = run_bass_kernel_spmd(
            nc, in_maps, core_ids=list(range(NCORES)), trace=trace
        )
    out = np.concatenate(
        [
            res.results[core]["out"]
            .reshape(P, BT, O)
            .transpose(1, 0, 2)
            .reshape(BC, O)
            for core in range(NCORES)
        ],
        axis=0,
    )
    return out, res


def kernel(x, weights, coeff):
    out, _ = _run(x, weights, coeff, trace=False)
    return out


def kernel_traced(x, weights, coeff):
    _install_ntff_hook()
    out, res = _run(x, weights, coeff, trace=True)
    return out, res



# revision 7
# speedup vs baseline: 1.1418x; 1.1418x over previous
"""Trainium2 Bass kernel for a custom Jacobi-basis layer.

Math:
    t = tanh(x)                                  x: [B, I] f32
    J[b,i,k] = P_k^(1,1)(t[b,i])                 Jacobi polys, k = 0..8
    out[b,o] = sum_{i,k} J[b,i,k] * coeff[o,i,k] * weights[o,i]

Strategy (8 NeuronCores, data-parallel over batch):
  * Fold weights into coeff on host: Cw[o,i,k] = coeff[o,i,k]*weights[o,i].
  * Replace the Jacobi basis with a Chebyshev-like basis phi_m(t) that is
    generable almost entirely on the Scalar (ACT) engine:
        phi1 = t            (tanh)
        phi2 = (sqrt2*t)^2          = 2t^2            [ACT square]
        phi3 = (phi2-1.5)*t                           [DVE stt]
        phi4 = (phi2-1.5)^2                           [ACT square]
        phi5 = (phi2-1.0)*phi3                        [DVE stt]
        phi6 = (2*phi3)^2                             [ACT square]
        phi7 = (phi4-0.75)*phi3                       [DVE stt]
        phi8 = (phi4-1.1)^2                           [ACT square]
    The exact change of basis J_k = sum_m C[k,m] phi_m is folded into the
    matmul operand on host (f64 poly algebra).  The phi's track scaled
    Chebyshev polynomials, so the folded operand R'_m stays O(1) and the
    fp16 quantization error is ~3e-3 (vs 1.5e-2 for a raw monomial basis).
    This removes the serial fp32 recurrence + per-plane fp16 casts that
    made DVE/ACT each ~30us busy; now ACT ~6us, DVE ~4us, and the kernel
    is purely PE-bound (128 fp16 [128x128]@[128x512] matmuls ~= 27.6us).
  * m=0 (constant) term becomes a per-output bias applied with one K=2
    matmul per b-tile (hi/lo fp16 split of the bias for accuracy), placed
    after the plane-1 matmuls so it is off the critical path.
  * x is shipped as fp16 (halves the gating transfer), the first r plane
    is split into 4 ic-chunks, and DMA issue is spread across the Sync /
    GpSimd / Vector queues so the first real matmul can start ~8us in.
  * Output is written fp16 (halves the tail DMA); host upcasts to f32.
"""

import numpy as np

import concourse.mybir as mybir
import concourse.tile as tile
from concourse import bacc
from concourse.bass_utils import run_bass_kernel_spmd

ORDER = 8
B, I, O = 4096, 512, 512
NCORES = 8
BC = B // NCORES          # batch rows per core = 512
P = 128                   # partitions
NIC = I // P              # i-chunks = 4
BT = BC // P              # b-tiles per core = 4
FREE = NIC * BC           # free dim of basis planes = 2048
SQRT2 = 1.4142135623730951

# basis shaping constants (see docstring); values chosen so phi_m ~ O(1)
C3 = -1.5   # phi3 = (phi2 + C3) * t,   phi4 = (phi2 + C3)^2
C5 = -1.0   # phi5 = (phi2 + C5) * phi3
C7 = -0.75  # phi7 = (phi4 + C7) * phi3
C8 = -1.1   # phi8 = (phi4 + C8)^2


def _basis_change():
    """Exact matrix C with J_k(t) = sum_m C[k,m] phi_m(t), f64 poly algebra."""
    Pp = np.polynomial.polynomial
    a = b = 1.0
    p1 = np.array([0.0, 1.0])
    p2 = Pp.polymul([0.0, SQRT2], [0.0, SQRT2])
    p2s = Pp.polyadd(p2, [C3])
    p3 = Pp.polymul(p2s, p1)
    p4 = Pp.polymul(p2s, p2s)
    p5 = Pp.polymul(Pp.polyadd(p2, [C5]), p3)
    p6 = Pp.polymul(Pp.polymul([2.0], p3), Pp.polymul([2.0], p3))
    p7 = Pp.polymul(Pp.polyadd(p4, [C7]), p3)
    p8 = Pp.polymul(Pp.polyadd(p4, [C8]), Pp.polyadd(p4, [C8]))
    basis = [np.array([1.0]), p1, p2, p3, p4, p5, p6, p7, p8]
    Mb = np.zeros((9, 9))
    for m, p in enumerate(basis):
        Mb[m, : len(p)] = p
    polys = [np.array([1.0]), np.array([0.0, 2.0])]
    for i in range(2, ORDER + 1):
        k1 = (2 * i + a + b) * (2 * i + a + b - 1) / (2 * i * (i + a + b))
        k3 = (i + a - 1) * (i + b - 1) * (2 * i + a + b) / (
            i * (i + a + b) * (2 * i + a + b - 2)
        )
        polys.append(
            Pp.polysub(Pp.polymul([0.0, k1], polys[-1]), Pp.polymul([k3], polys[-2]))
        )
    MJ = np.zeros((9, 9))
    for k, p in enumerate(polys):
        MJ[k, : len(p)] = p
    return MJ @ np.linalg.inv(Mb)


def _build_module():
    nc = bacc.Bacc("TRN2", num_devices=NCORES)
    f32 = mybir.dt.float32
    f16 = mybir.dt.float16

    # xt chunk-major: [ic, p, BC]; xt[ic, p, b] = x[core*BC+b, ic*128+p], fp16
    xt_d = nc.dram_tensor("xt", [NIC, P, BC], f16, kind="ExternalInput")
    # r layout: [p, (m-1)*FREE + ic*O + o] = R'_m[o, ic*128+p], fp16
    r_d = nc.dram_tensor("r", [P, ORDER * FREE], f16, kind="ExternalInput")
    # consts rows: [ones(128) | bias_hi(512)] ; [ones(128) | bias_lo(512)]
    consts_d = nc.dram_tensor("consts", [2, P + O], f16, kind="ExternalInput")
    # out layout: [p, bt*O + o] = output[core*BC + bt*128 + p, o], fp16
    out_d = nc.dram_tensor("out", [P, BT * O], f16, kind="ExternalOutput")

    mult = mybir.AluOpType.mult
    add = mybir.AluOpType.add
    Square = mybir.ActivationFunctionType.Square
    Tanh = mybir.ActivationFunctionType.Tanh

    from concourse.tile_rust import add_dep_helper

    H = FREE // 2
    halves = (slice(0, H), slice(H, FREE))

    with tile.TileContext(nc) as tc:
        with (
            tc.tile_pool(name="io", bufs=1) as io,
            tc.tile_pool(name="psum", bufs=1, space="PSUM") as pp,
        ):
            def ics(ap, ic):
                return ap[:, ic * BC : (ic + 1) * BC]

            # --- small consts for ACT square biases (gpsimd, ~instant) ---
            c3_t = io.tile([P, 1], f32, tag="c3")
            c8_t = io.tile([P, 1], f32, tag="c8")
            nc.gpsimd.memset(c3_t[:], C3)
            nc.gpsimd.memset(c8_t[:], C8)

            # --- input DMAs, spread across engine queues ---
            x_t = io.tile([P, FREE], f16, tag="x")
            r_t = [io.tile([P, FREE], f16, tag=f"r{m}", name=f"r{m}")
                   for m in range(1, ORDER + 1)]
            const_t = io.tile([2, P + O], f16, tag="consts")

            def rsl(m, lo, hi):
                return r_d[:, (m - 1) * FREE + lo : (m - 1) * FREE + hi]

            # Sync queue: xt chunks, consts, then r5..r8 (dep-laddered so
            # the late planes don't steal HBM bandwidth from the early ones).
            for ic in range(NIC):
                nc.sync.dma_start(ics(x_t, ic), xt_d[ic])
            nc.sync.dma_start(const_t[:], consts_d[:])
            # GpSimd queue: r1 in 4 ic-chunks (gates the first matmuls), then
            # r2..r4 whole.
            d_r1 = []
            for ic in range(NIC):
                d_r1.append(nc.gpsimd.dma_start(
                    r_t[0][:, ic * O : (ic + 1) * O],
                    rsl(1, ic * O, (ic + 1) * O)))
            d_r2 = nc.gpsimd.dma_start(r_t[1][:], rsl(2, 0, FREE))
            d_r3 = nc.gpsimd.dma_start(r_t[2][:], rsl(3, 0, FREE))
            d_r4 = nc.gpsimd.dma_start(r_t[3][:], rsl(4, 0, FREE))
            # Vector queue: junk-warmup memset (gates the PE warmup matmuls).
            junk_t = io.tile([P, P + O], f16, tag="junk")
            nc.vector.memset(junk_t[:], 0.25)
            # Sync queue tail: r5..r8 paced behind r1..r4's transfers.
            for m, gate in ((5, d_r1[3]), (6, d_r2), (7, d_r3), (8, d_r4)):
                d = nc.sync.dma_start(r_t[m - 1][:], rsl(m, 0, FREE))
                add_dep_helper(d.ins, gate.ins, reason="dma ladder")

            # --- PE warmup: junk matmuls release the HAM clock gate while
            # the tanh / r1 pipeline fills ---
            ps_warm = pp.tile([P, O], f32, tag="warm", name="ps_warm")
            for _ in range(5):
                nc.tensor.matmul(
                    ps_warm[:], junk_t[:, 0:P], junk_t[:, P : P + O],
                    start=True, stop=True,
                )

            # --- basis planes (all fp16) ---
            ph = [None] * (ORDER + 1)
            for m in range(1, ORDER + 1):
                ph[m] = io.tile([P, FREE], f16, tag=f"ph{m}", name=f"ph{m}")
            t16 = ph[1]
            # Scalar queue: tanh per ic-chunk (pipeline start), squares per half
            for ic in range(NIC):
                nc.scalar.activation(ics(t16, ic), ics(x_t, ic), Tanh)
            for h in (0, 1):
                sl = halves[h]
                nc.scalar.activation(ph[2][:, sl], t16[:, sl], Square, scale=SQRT2)
            for h in (0, 1):
                sl = halves[h]
                nc.vector.scalar_tensor_tensor(
                    ph[3][:, sl], ph[2][:, sl], C3, t16[:, sl], add, mult)
            for h in (0, 1):
                sl = halves[h]
                nc.scalar.activation(ph[4][:, sl], ph[2][:, sl], Square,
                                     bias=c3_t[:])
            for h in (0, 1):
                sl = halves[h]
                nc.vector.scalar_tensor_tensor(
                    ph[5][:, sl], ph[2][:, sl], C5, ph[3][:, sl], add, mult)
            for h in (0, 1):
                sl = halves[h]
                nc.scalar.activation(ph[6][:, sl], ph[3][:, sl], Square, scale=2.0)
            for h in (0, 1):
                sl = halves[h]
                nc.vector.scalar_tensor_tensor(
                    ph[7][:, sl], ph[4][:, sl], C7, ph[3][:, sl], add, mult)
            for h in (0, 1):
                sl = halves[h]
                nc.scalar.activation(ph[8][:, sl], ph[4][:, sl], Square,
                                     bias=c8_t[:])

            # --- matmul stream: psum[bt] += sum_{m,ic} phi_m_blk^T @ R'_m_blk
            psums = [
                pp.tile([P, O], f32, tag=f"ps{bt}", name=f"ps{bt}")
                for bt in range(BT)
            ]
            out_t = io.tile([P, BT * O], f16, tag="out")
            # plane 1 opens each psum group
            for ic in range(NIC):
                for bt in range(BT):
                    col = ic * BC + bt * P
                    nc.tensor.matmul(
                        psums[bt][:], ph[1][:, col : col + P],
                        r_t[0][:, ic * O : (ic + 1) * O],
                        start=ic == 0, stop=False,
                    )
            # bias (K=2 hi/lo) — consts arrived long ago; off the hot path
            for bt in range(BT):
                nc.tensor.matmul(
                    psums[bt][:], const_t[:, 0:P], const_t[:, P : P + O],
                    start=False, stop=False,
                )
            for m in range(2, ORDER + 1):
                if m < ORDER:
                    for ic in range(NIC):
                        for bt in range(BT):
                            col = ic * BC + bt * P
                            nc.tensor.matmul(
                                psums[bt][:], ph[m][:, col : col + P],
                                r_t[m - 1][:, ic * O : (ic + 1) * O],
                                start=False, stop=False,
                            )
                else:
                    # last plane bt-major: each b-tile finishes early so its
                    # psum copy + out DMA overlap the remaining matmuls
                    for bt in range(BT):
                        for ic in range(NIC):
                            col = ic * BC + bt * P
                            nc.tensor.matmul(
                                psums[bt][:], ph[m][:, col : col + P],
                                r_t[m - 1][:, ic * O : (ic + 1) * O],
                                start=False, stop=ic == NIC - 1,
                            )
                        dst = out_t[:, bt * O : (bt + 1) * O]
                        if bt % 2 == 0:
                            nc.scalar.copy(dst, psums[bt][:])
                        else:
                            nc.vector.tensor_copy(dst, psums[bt][:])
                        nc.sync.dma_start(
                            out_d[:, bt * O : (bt + 1) * O], dst,
                        )
    nc.compile()
    return nc


def _prep_operands(weights, coeff):
    """Host-side, input-independent preprocessing of the layer constants."""
    C = _basis_change()
    Cw = coeff.astype(np.float64) * weights.astype(np.float64)[:, :, None]
    Rm = np.einsum("oik,km->oim", Cw, C)            # [O, I, 9] in phi basis
    bias = Rm[:, :, 0].sum(axis=1)                  # [O]
    b_hi = bias.astype(np.float16)
    b_lo = (bias - b_hi.astype(np.float64)).astype(np.float16)
    r = np.empty((ORDER, P, FREE), dtype=np.float32)
    for m in range(1, ORDER + 1):
        tmp = Rm[:, :, m].T.astype(np.float32)       # [I, O]
        r[m - 1] = tmp.reshape(NIC, P, O).transpose(1, 0, 2).reshape(P, FREE)
    r = np.ascontiguousarray(
        r.transpose(1, 0, 2).reshape(P, ORDER * FREE)
    ).astype(np.float16)
    consts = np.ones((2, P + O), dtype=np.float16)
    consts[0, P:] = b_hi
    consts[1, P:] = b_lo
    return r, consts


def _prep_x(x):
    """Per-core [NIC, 128, BC] fp16 views: xt[ic, p, b] = x[core*BC+b, ic*128+p]."""
    shards = []
    for core in range(NCORES):
        xc = np.ascontiguousarray(
            x[core * BC : (core + 1) * BC, :].T.astype(np.float16)
        )  # [I, BC]
        shards.append(np.ascontiguousarray(xc.reshape(NIC, P, BC)))
    return shards


def _install_ntff_hook():
    """Register the NTFF profile hook that the image's boot skips (no
    antenv.axon_hooks module). Same ctypes ABI as trn_boot's
    _ntff_profile_via_ctypes. Only used for traced (profiling) runs."""
    import sys
    import types
    import ctypes
    import contextlib

    if "antenv.axon_hooks" in sys.modules:
        return
    mod = types.ModuleType("antenv.axon_hooks")
    state = {"hook": None}
    mod.set_axon_ntff_profile_hook = lambda h: state.__setitem__("hook", h)
    mod.get_axon_ntff_profile_hook = lambda: state["hook"]
    sys.modules["antenv.axon_hooks"] = mod
    import antenv

    antenv.axon_hooks = mod

    so_path = "/opt/axon/libaxon_pjrt.so"
    lib = ctypes.CDLL(so_path)
    if not hasattr(lib, "axon_start_nrt_profile"):
        return
    lib.axon_start_nrt_profile.argtypes = [
        ctypes.POINTER(ctypes.c_int64),
        ctypes.c_size_t,
    ]
    lib.axon_start_nrt_profile.restype = ctypes.c_int64
    lib.axon_stop_nrt_profile.argtypes = [ctypes.c_char_p]
    lib.axon_stop_nrt_profile.restype = ctypes.c_int64

    @contextlib.contextmanager
    def _hook(output_dir, device_ids):
        import jax

        jax.devices()
        if device_ids:
            ids = (ctypes.c_int64 * len(device_ids))(*device_ids)
            rc = lib.axon_start_nrt_profile(ids, len(device_ids))
        else:
            rc = lib.axon_start_nrt_profile(None, 0)
        if rc != 0:
            raise RuntimeError(f"axon_start_nrt_profile rc={rc}")
        try:
            yield
        finally:
            n = lib.axon_stop_nrt_profile(str(output_dir).encode())
            print(f"ntff profile: {n} file(s) written to {output_dir}")

    mod.set_axon_ntff_profile_hook(_hook)


_NC_CACHE = None


def _get_module():
    global _NC_CACHE
    if _NC_CACHE is None:
        _NC_CACHE = _build_module()
    return _NC_CACHE


def _run(x, weights, coeff, trace=False):
    nc = _get_module()
    r, consts = _prep_operands(weights, coeff)
    xs = _prep_x(np.asarray(x, dtype=np.float32))
    in_maps = [
        {"xt": xs[core], "r": r, "consts": consts} for core in range(NCORES)
    ]
    try:
        res = run_bass_kernel_spmd(
            nc, in_maps, core_ids=list(range(NCORES)), trace=trace
        )
    except Exception:
        res = run_bass_kernel_spmd(
            nc, in_maps, core_ids=list(range(NCORES)), trace=trace
        )
    out = np.concatenate(
        [
            res.results[core]["out"]
            .astype(np.float32)
            .reshape(P, BT, O)
            .transpose(1, 0, 2)
            .reshape(BC, O)
            for core in range(NCORES)
        ],
        axis=0,
    )
    return out, res


def kernel(x, weights, coeff):
    out, _ = _run(x, weights, coeff, trace=False)
    return out


def kernel_traced(x, weights, coeff):
    _install_ntff_hook()
    out, res = _run(x, weights, coeff, trace=True)
    return out, res


# revision 9
# speedup vs baseline: 1.1455x; 1.0032x over previous
"""Trainium2 Bass kernel for a custom Jacobi-basis layer.

Math:
    t = tanh(x)                                  x: [B, I] f32
    J[b,i,k] = P_k^(1,1)(t[b,i])                 Jacobi polys, k = 0..8
    out[b,o] = sum_{i,k} J[b,i,k] * coeff[o,i,k] * weights[o,i]

Strategy (8 NeuronCores, data-parallel over batch):
  * Fold weights into coeff on host: Cw[o,i,k] = coeff[o,i,k]*weights[o,i].
  * Replace the Jacobi basis with a Chebyshev-like basis phi_m(t) that is
    generable almost entirely on the Scalar (ACT) engine:
        phi1 = t            (tanh)
        phi2 = (sqrt2*t)^2          = 2t^2            [ACT square]
        phi3 = (phi2-1.5)*t                           [DVE stt]
        phi4 = (phi2-1.5)^2                           [ACT square]
        phi5 = (phi2-1.0)*phi3                        [DVE stt]
        phi6 = (2*phi3)^2                             [ACT square]
        phi7 = (phi4-0.75)*phi3                       [DVE stt]
        phi8 = (phi4-1.1)^2                           [ACT square]
    The exact change of basis J_k = sum_m C[k,m] phi_m is folded into the
    matmul operand on host (f64 poly algebra).  The phi's track scaled
    Chebyshev polynomials, so the folded operand R'_m stays O(1) and the
    fp16 quantization error is ~3e-3 (vs 1.5e-2 for a raw monomial basis).
    This removes the serial fp32 recurrence + per-plane fp16 casts that
    made DVE/ACT each ~30us busy; now ACT ~6us, DVE ~4us, and the kernel
    is purely PE-bound (128 fp16 [128x128]@[128x512] matmuls ~= 27.6us).
  * The m=0 (constant) term is a per-output bias that is independent of x;
    it is added on the host after the gather (saves 4 PE matmuls and the
    consts DMA).
  * Pre-tile raw warmup: a dummy DMA on each hardware DMA ring absorbs the
    ~0.9us ring spin-up, a dummy activation hoists the 1.3us ACT-table
    load before the tile entry gate, and junk matmuls (raw + in-tile)
    release the PE HAM clock gate before the real stream begins.
  * x is shipped as fp16, the first r plane is split into 4 ic-chunks, and
    DMA issue is spread across the Sync / GpSimd queues.
  * Planes 7 and 8 run b-tile-major so each b-tile's psum finishes early;
    psum->out copies are split across Scalar/Vector and the fp16 output
    chunks stream on both DMA rings while the last matmuls still run.
    Host upcasts the fp16 output to f32.
"""

import numpy as np

import concourse.mybir as mybir
import concourse.tile as tile
from concourse import bacc
from concourse.bass_utils import run_bass_kernel_spmd

ORDER = 8
B, I, O = 4096, 512, 512
NCORES = 8
BC = B // NCORES          # batch rows per core = 512
P = 128                   # partitions
NIC = I // P              # i-chunks = 4
BT = BC // P              # b-tiles per core = 4
FREE = NIC * BC           # free dim of basis planes = 2048
SQRT2 = 1.4142135623730951

# basis shaping constants (see docstring); values chosen so phi_m ~ O(1)
C3 = -1.5   # phi3 = (phi2 + C3) * t,   phi4 = (phi2 + C3)^2
C5 = -1.0   # phi5 = (phi2 + C5) * phi3
C7 = -0.75  # phi7 = (phi4 + C7) * phi3
C8 = -1.1   # phi8 = (phi4 + C8)^2


def _basis_change():
    """Exact matrix C with J_k(t) = sum_m C[k,m] phi_m(t), f64 poly algebra."""
    Pp = np.polynomial.polynomial
    a = b = 1.0
    p1 = np.array([0.0, 1.0])
    p2 = Pp.polymul([0.0, SQRT2], [0.0, SQRT2])
    p2s = Pp.polyadd(p2, [C3])
    p3 = Pp.polymul(p2s, p1)
    p4 = Pp.polymul(p2s, p2s)
    p5 = Pp.polymul(Pp.polyadd(p2, [C5]), p3)
    p6 = Pp.polymul(Pp.polymul([2.0], p3), Pp.polymul([2.0], p3))
    p7 = Pp.polymul(Pp.polyadd(p4, [C7]), p3)
    p8 = Pp.polymul(Pp.polyadd(p4, [C8]), Pp.polyadd(p4, [C8]))
    basis = [np.array([1.0]), p1, p2, p3, p4, p5, p6, p7, p8]
    Mb = np.zeros((9, 9))
    for m, p in enumerate(basis):
        Mb[m, : len(p)] = p
    polys = [np.array([1.0]), np.array([0.0, 2.0])]
    for i in range(2, ORDER + 1):
        k1 = (2 * i + a + b) * (2 * i + a + b - 1) / (2 * i * (i + a + b))
        k3 = (i + a - 1) * (i + b - 1) * (2 * i + a + b) / (
            i * (i + a + b) * (2 * i + a + b - 2)
        )
        polys.append(
            Pp.polysub(Pp.polymul([0.0, k1], polys[-1]), Pp.polymul([k3], polys[-2]))
        )
    MJ = np.zeros((9, 9))
    for k, p in enumerate(polys):
        MJ[k, : len(p)] = p
    return MJ @ np.linalg.inv(Mb)


def _build_module():
    nc = bacc.Bacc("TRN2", num_devices=NCORES)
    f32 = mybir.dt.float32
    f16 = mybir.dt.float16

    # xt chunk-major: [ic, p, BC]; xt[ic, p, b] = x[core*BC+b, ic*128+p], fp16
    xt_d = nc.dram_tensor("xt", [NIC, P, BC], f16, kind="ExternalInput")
    # r layout: [p, (m-1)*FREE + ic*O + o] = R'_m[o, ic*128+p], fp16
    r_d = nc.dram_tensor("r", [P, ORDER * FREE], f16, kind="ExternalInput")
    # out layout: [p, bt*O + o] = unbiased output[core*BC + bt*128 + p, o], fp16
    out_d = nc.dram_tensor("out", [P, BT * O], f16, kind="ExternalOutput")

    mult = mybir.AluOpType.mult
    add = mybir.AluOpType.add
    Square = mybir.ActivationFunctionType.Square
    Tanh = mybir.ActivationFunctionType.Tanh

    from concourse.tile_rust import add_dep_helper

    H = FREE // 2
    halves = (slice(0, H), slice(H, FREE))

    # ---- raw pre-tile warmup (the tile entry gate orders all of this
    # before the tile body; contents of the scratch data are irrelevant) ----
    # 1. hoist the ACT-table load (1.3us) before the tile gate
    act_scr = nc.alloc_sbuf_tensor("act_scr", [P, 1], f32)
    nc.scalar.activation(act_scr.ap(), act_scr.ap(), Tanh)
    # 2. absorb the DMA-ring spin-up on both hardware rings
    dma_scr = nc.alloc_sbuf_tensor("dma_scr", [2, 64], f16)
    warm_sem = nc.alloc_semaphore("warm_dma_sem")
    nc.sync.dma_start(dma_scr.ap()[0:1], r_d[0:1, 0:64]).then_inc(warm_sem, 16)
    nc.gpsimd.dma_start(dma_scr.ap()[1:2], r_d[1:2, 0:64]).then_inc(warm_sem, 16)
    # 3. start the PE HAM busy-window early with junk matmuls
    junk_sb = nc.alloc_sbuf_tensor("junk_sb", [P, P + O], f16)
    ps_junk = nc.alloc_psum_tensor("ps_junk", [P, O], f32)
    for _ in range(2):
        nc.tensor.matmul(
            ps_junk.ap(), junk_sb.ap()[:, 0:P], junk_sb.ap()[:, P : P + O],
            start=True, stop=True,
        )

    with tile.TileContext(nc) as tc:
        with (
            tc.tile_pool(name="io", bufs=1) as io,
            tc.tile_pool(name="psum", bufs=1, space="PSUM") as pp,
        ):
            def ics(ap, ic):
                return ap[:, ic * BC : (ic + 1) * BC]

            # --- small consts for ACT square biases (gpsimd, ~instant) ---
            c3_t = io.tile([P, 1], f32, tag="c3")
            c8_t = io.tile([P, 1], f32, tag="c8")
            nc.gpsimd.memset(c3_t[:], C3)
            nc.gpsimd.memset(c8_t[:], C8)

            # --- input DMAs, spread across the two hardware rings ---
            x_t = io.tile([P, FREE], f16, tag="x")
            r_t = [io.tile([P, FREE], f16, tag=f"r{m}", name=f"r{m}")
                   for m in range(1, ORDER + 1)]

            def rsl(m, lo, hi):
                return r_d[:, (m - 1) * FREE + lo : (m - 1) * FREE + hi]

            # Sync queue/ring: xt chunks, then r5..r8 (dep-laddered so the
            # late planes don't steal HBM bandwidth from the early ones).
            for ic in range(NIC):
                nc.sync.dma_start(ics(x_t, ic), xt_d[ic])
            # GpSimd queue/ring: r1 in 4 ic-chunks (gates the first matmuls),
            # then r2..r4 whole.
            d_r1 = []
            for ic in range(NIC):
                d_r1.append(nc.gpsimd.dma_start(
                    r_t[0][:, ic * O : (ic + 1) * O],
                    rsl(1, ic * O, (ic + 1) * O)))
            d_r2 = nc.gpsimd.dma_start(r_t[1][:], rsl(2, 0, FREE))
            d_r3 = nc.gpsimd.dma_start(r_t[2][:], rsl(3, 0, FREE))
            d_r4 = nc.gpsimd.dma_start(r_t[3][:], rsl(4, 0, FREE))
            for m, gate in ((5, d_r1[3]), (6, d_r2), (7, d_r3), (8, d_r4)):
                d = nc.sync.dma_start(r_t[m - 1][:], rsl(m, 0, FREE))
                add_dep_helper(d.ins, gate.ins, reason="dma ladder")

            # --- in-tile junk matmuls bridge the HAM busy-window from the
            # raw warmup to the first real matmul (~2.5us of queue wait) ---
            for _ in range(5):
                nc.tensor.matmul(
                    ps_junk.ap(), junk_sb.ap()[:, 0:P], junk_sb.ap()[:, P : P + O],
                    start=True, stop=True,
                )

            # --- basis planes (all fp16) ---
            ph = [None] * (ORDER + 1)
            for m in range(1, ORDER + 1):
                ph[m] = io.tile([P, FREE], f16, tag=f"ph{m}", name=f"ph{m}")
            t16 = ph[1]
            for ic in range(NIC):
                nc.scalar.activation(ics(t16, ic), ics(x_t, ic), Tanh)
            for h in (0, 1):
                sl = halves[h]
                nc.scalar.activation(ph[2][:, sl], t16[:, sl], Square, scale=SQRT2)
            for h in (0, 1):
                sl = halves[h]
                nc.vector.scalar_tensor_tensor(
                    ph[3][:, sl], ph[2][:, sl], C3, t16[:, sl], add, mult)
            for h in (0, 1):
                sl = halves[h]
                nc.scalar.activation(ph[4][:, sl], ph[2][:, sl], Square,
                                     bias=c3_t[:])
            for h in (0, 1):
                sl = halves[h]
                nc.vector.scalar_tensor_tensor(
                    ph[5][:, sl], ph[2][:, sl], C5, ph[3][:, sl], add, mult)
            for h in (0, 1):
                sl = halves[h]
                nc.scalar.activation(ph[6][:, sl], ph[3][:, sl], Square, scale=2.0)
            for h in (0, 1):
                sl = halves[h]
                nc.vector.scalar_tensor_tensor(
                    ph[7][:, sl], ph[4][:, sl], C7, ph[3][:, sl], add, mult)
            for h in (0, 1):
                sl = halves[h]
                nc.scalar.activation(ph[8][:, sl], ph[4][:, sl], Square,
                                     bias=c8_t[:])

            # --- matmul stream: psum[bt] += sum_{m,ic} phi_m_blk^T @ R'_m_blk
            psums = [
                pp.tile([P, O], f32, tag=f"ps{bt}", name=f"ps{bt}")
                for bt in range(BT)
            ]
            out_t = io.tile([P, BT * O], f16, tag="out")
            for m in range(1, ORDER - 1):
                for ic in range(NIC):
                    for bt in range(BT):
                        col = ic * BC + bt * P
                        nc.tensor.matmul(
                            psums[bt][:], ph[m][:, col : col + P],
                            r_t[m - 1][:, ic * O : (ic + 1) * O],
                            start=(m == 1 and ic == 0), stop=False,
                        )
            # last two planes b-tile-major: each b-tile's psum finishes ~1.7us
            # apart, so copies + out DMA stream under the remaining matmuls
            HO = O // 2
            for bt in range(BT):
                for m in (ORDER - 1, ORDER):
                    for ic in range(NIC):
                        col = ic * BC + bt * P
                        nc.tensor.matmul(
                            psums[bt][:], ph[m][:, col : col + P],
                            r_t[m - 1][:, ic * O : (ic + 1) * O],
                            start=False, stop=(m == ORDER and ic == NIC - 1),
                        )
                lo = bt * O
                # split the psum->fp16 copy across Scalar and Vector, and the
                # out chunk across both DMA rings
                nc.scalar.copy(out_t[:, lo : lo + HO], psums[bt][:, 0:HO])
                nc.vector.tensor_copy(out_t[:, lo + HO : lo + O], psums[bt][:, HO:O])
                nc.sync.dma_start(out_d[:, lo : lo + HO], out_t[:, lo : lo + HO])
                nc.gpsimd.dma_start(
                    out_d[:, lo + HO : lo + O], out_t[:, lo + HO : lo + O])
    nc.compile()
    return nc


def _prep_operands(weights, coeff):
    """Host-side, input-independent preprocessing of the layer constants."""
    C = _basis_change()
    Cw = coeff.astype(np.float64) * weights.astype(np.float64)[:, :, None]
    Rm = np.einsum("oik,km->oim", Cw, C)            # [O, I, 9] in phi basis
    bias = Rm[:, :, 0].sum(axis=1).astype(np.float32)   # [O], added on host
    r = np.empty((ORDER, P, FREE), dtype=np.float32)
    for m in range(1, ORDER + 1):
        tmp = Rm[:, :, m].T.astype(np.float32)       # [I, O]
        r[m - 1] = tmp.reshape(NIC, P, O).transpose(1, 0, 2).reshape(P, FREE)
    r = np.ascontiguousarray(
        r.transpose(1, 0, 2).reshape(P, ORDER * FREE)
    ).astype(np.float16)
    return r, bias


def _prep_x(x):
    """Per-core [NIC, 128, BC] fp16 views: xt[ic, p, b] = x[core*BC+b, ic*128+p]."""
    shards = []
    for core in range(NCORES):
        xc = np.ascontiguousarray(
            x[core * BC : (core + 1) * BC, :].T.astype(np.float16)
        )  # [I, BC]
        shards.append(np.ascontiguousarray(xc.reshape(NIC, P, BC)))
    return shards


def _install_ntff_hook():
    """Register the NTFF profile hook that the image's boot skips (no
    antenv.axon_hooks module). Same ctypes ABI as trn_boot's
    _ntff_profile_via_ctypes. Only used for traced (profiling) runs."""
    import sys
    import types
    import ctypes
    import contextlib

    if "antenv.axon_hooks" in sys.modules:
        return
    mod = types.ModuleType("antenv.axon_hooks")
    state = {"hook": None}
    mod.set_axon_ntff_profile_hook = lambda h: state.__setitem__("hook", h)
    mod.get_axon_ntff_profile_hook = lambda: state["hook"]
    sys.modules["antenv.axon_hooks"] = mod
    import antenv

    antenv.axon_hooks = mod

    so_path = "/opt/axon/libaxon_pjrt.so"
    lib = ctypes.CDLL(so_path)
    if not hasattr(lib, "axon_start_nrt_profile"):
        return
    lib.axon_start_nrt_profile.argtypes = [
        ctypes.POINTER(ctypes.c_int64),
        ctypes.c_size_t,
    ]
    lib.axon_start_nrt_profile.restype = ctypes.c_int64
    lib.axon_stop_nrt_profile.argtypes = [ctypes.c_char_p]
    lib.axon_stop_nrt_profile.restype = ctypes.c_int64

    @contextlib.contextmanager
    def _hook(output_dir, device_ids):
        import jax

        jax.devices()
        if device_ids:
            ids = (ctypes.c_int64 * len(device_ids))(*device_ids)
            rc = lib.axon_start_nrt_profile(ids, len(device_ids))
        else:
            rc = lib.axon_start_nrt_profile(None, 0)
        if rc != 0:
            raise RuntimeError(f"axon_start_nrt_profile rc={rc}")
        try:
            yield
        finally:
            n = lib.axon_stop_nrt_profile(str(output_dir).encode())
            print(f"ntff profile: {n} file(s) written to {output_dir}")

    mod.set_axon_ntff_profile_hook(_hook)


_NC_CACHE = None


def _get_module():
    global _NC_CACHE
    if _NC_CACHE is None:
        _NC_CACHE = _build_module()
    return _NC_CACHE


def _run(x, weights, coeff, trace=False):
    nc = _get_module()
    r, bias = _prep_operands(weights, coeff)
    xs = _prep_x(np.asarray(x, dtype=np.float32))
    in_maps = [{"xt": xs[core], "r": r} for core in range(NCORES)]
    try:
        res = run_bass_kernel_spmd(
            nc, in_maps, core_ids=list(range(NCORES)), trace=trace
        )
    except Exception:
        res = run_bass_kernel_spmd(
            nc, in_maps, core_ids=list(range(NCORES)), trace=trace
        )
    out = np.concatenate(
        [
            res.results[core]["out"]
            .astype(np.float32)
            .reshape(P, BT, O)
            .transpose(1, 0, 2)
            .reshape(BC, O)
            for core in range(NCORES)
        ],
        axis=0,
    )
    out += bias[None, :]
    return out, res


def kernel(x, weights, coeff):
    out, _ = _run(x, weights, coeff, trace=False)
    return out


def kernel_traced(x, weights, coeff):
    _install_ntff_hook()
    out, res = _run(x, weights, coeff, trace=True)
    return out, res
